# revision 33
# baseline (speedup 1.0000x reference)
"""Causal multi-head attention with RoPE for Trainium2, sharded over 8 NeuronCores.

Problem: B=4, T=2048, C=768, H=12, D=64, fp32 in/out.
    q,k,v = x @ wq/wk/wv  (per-head reshape), RoPE(q,k), causal softmax(q k^T/sqrt(D)) v,
    out = concat_heads @ wo.

Sharding: core c -> (batch b = c//2, head-group g = c%2 covering heads g*6..g*6+5).
Each core computes its 6 heads' attention and a partial output projection
y_c = out_heads(g) @ wo[rows g]; the host sums the two partials per batch.

PE-saturation schedule.  The PE is the serial resource (~135us of matmul
column-drain at 1 col/cycle, measured ~183us busy incl. weight loads);
the ScalarE exp stream is ~125us (0.75ns/col + ~330ns/instr).  Everything
is organized to keep the PE warm and dense from ~9us to the end:
  - inputs split across the 3 DMA queues (sync/scalar HWDGE + gpsimd SWDGE),
    whole-tensor weight loads (narrow column slices DMA ~5x slower), x^T
    behind wq/wk, RoPE tables last; the input load is device-HBM-bound and
    completes ~27us, bridged by a warmup matmul stream on r2t.
  - phase 1 computes only pair-0 k,q, consuming x chunks in DMA arrival
    order; per-hh waves let the PSUM->SBUF copies overlap the next half's
    matmuls, and the first exp fires ~30us instead of ~40us.
  - pairs 1-2 projections (as per-512-col quarter fillers through the shared
    1-bank aux PSUM ring), the v-projection, and the output projections all
    run as PE filler *inside* the attention score/exp/PV pipeline, paced so
    each stretch's dependencies are ready just in time and the PE rarely
    idles long enough to re-throttle (HAM 3.4us window).
  - attention per (pair, qc): S^T tiles k-paired on 64-row groups, exp on
    ScalarE with 2-kc-wide per-head tiles (finer tiles pay the per-instr
    overhead, coarser merged-head tiles serialize the score/exp ping-pong),
    PV accumulated with an appended ones-row for the softmax denominator.
  - normalization is split: a fast PSUM drain + gpsimd broadcast at stretch
    end, with the reciprocal+multiply deferred into the next stretch so the
    PE never waits on the chain (gpsimd tensor ops are ~4x slower than DVE
    -- keep everything element-wise on DVE).
"""

import numpy as np
from contextlib import ExitStack

B, T, C, H, D = 4, 2048, 768, 12, 64
HPC = 6          # heads per core
NP = 3           # head-pair tiles per core
CC = C // 128    # 6 contraction chunks
TT = T // 128    # 16 t tiles
QC = T // 512    # 4 q chunks
KC = T // 128    # 16 k chunks

_COMPILED = None


def _rope_tables():
    import ml_dtypes
    inv_freq = 1.0 / (10000.0 ** (np.arange(0, D, 2, dtype=np.float64) / D))  # [32]
    t = np.arange(T, dtype=np.float64)
    freqs = np.outer(t, inv_freq)                      # [T, 32]
    cosT = np.cos(freqs).T.astype(np.float32)          # [32, T]
    sinT = np.sin(freqs).T.astype(np.float32)
    ccat = np.tile(cosT, (4, 1)).astype(ml_dtypes.bfloat16)   # [128, T]
    scat = np.tile(sinT, (4, 1)).astype(ml_dtypes.bfloat16)
    return np.ascontiguousarray(ccat), np.ascontiguousarray(scat)


def _rot_matrix():
    import ml_dtypes
    # rotate_half as a matmul: rot = R @ q (q in [D, T] layout), per 64-row block
    R = np.zeros((D, D), dtype=np.float32)
    R[0:32, 32:64] = -np.eye(32, dtype=np.float32)
    R[32:64, 0:32] = np.eye(32, dtype=np.float32)
    R2 = np.zeros((128, 128), dtype=np.float32)
    R2[0:64, 0:64] = R
    R2[64:128, 64:128] = R
    return np.ascontiguousarray(R2.T.astype(ml_dtypes.bfloat16))  # lhsT for out = R2 @ q


def _build_program():
    import concourse.tile as tile
    from concourse import bacc, mybir

    F32 = mybir.dt.float32
    BF16 = mybir.dt.bfloat16
    EXP = mybir.ActivationFunctionType.Exp

    nc = bacc.Bacc("TRN2", target_bir_lowering=False, debug=False, num_devices=8)

    xT_d = nc.dram_tensor("xT", [C, T], BF16, kind="ExternalInput").ap()
    wq_d = nc.dram_tensor("wq", [C, HPC * D], BF16, kind="ExternalInput").ap()
    wk_d = nc.dram_tensor("wk", [C, HPC * D], BF16, kind="ExternalInput").ap()
    wv_d = nc.dram_tensor("wv", [C, HPC * D], BF16, kind="ExternalInput").ap()
    wo_d = nc.dram_tensor("wo", [HPC * D, C], BF16, kind="ExternalInput").ap()
    ccat_d = nc.dram_tensor("ccat", [128, T], BF16, kind="ExternalInput").ap()
    scat_d = nc.dram_tensor("scat", [128, T], BF16, kind="ExternalInput").ap()
    r2t_d = nc.dram_tensor("r2t", [128, 128], BF16, kind="ExternalInput").ap()
    utri_d = nc.dram_tensor("utri", [128, 256], BF16, kind="ExternalInput").ap()
    y_d = nc.dram_tensor("y", [T, C], BF16, kind="ExternalOutput").ap()

    with tile.TileContext(nc) as tc, ExitStack() as ctx:
        big_pool = ctx.enter_context(tc.tile_pool(name="big", bufs=1))
        q_all = big_pool.tile([128, NP, T], BF16)
        k_all = big_pool.tile([128, NP, T], BF16)
        v_aug = big_pool.tile([128, KC, HPC, D + 1], BF16)
        out_norm = big_pool.tile([128, NP, T], BF16)

        cst_pool = ctx.enter_context(tc.tile_pool(name="cst", bufs=1))
        xt_pool = ctx.enter_context(tc.tile_pool(name="xt", bufs=1))
        xt_sb = xt_pool.tile([128, CC, T], BF16)

        p_sbp = ctx.enter_context(tc.tile_pool(name="p_sb", bufs=12))
        l_sbp = ctx.enter_context(tc.tile_pool(name="l_sb", bufs=3))
        r_sbp = ctx.enter_context(tc.tile_pool(name="r_sb", bufs=3))
        y_sbp = ctx.enter_context(tc.tile_pool(name="y_sb", bufs=2))
        qr_sbp = ctx.enter_context(tc.tile_pool(name="qr_sb", bufs=3))

        r2t = cst_pool.tile([128, 128], BF16)
        wq_sb = cst_pool.tile([128, CC, HPC * D], BF16)
        wk_sb = cst_pool.tile([128, CC, HPC * D], BF16)
        wv_sb = cst_pool.tile([128, CC, HPC * D], BF16)
        wo_sb = cst_pool.tile([128, NP, C], BF16)
        ccat = cst_pool.tile([128, T], BF16)
        scat = cst_pool.tile([128, T], BF16)
        utri2 = cst_pool.tile([128, 256], BF16)
        exp_warm = cst_pool.tile([1, 2], F32)

        # ------------- input DMAs: sync + scalar (HWDGE) + gpsimd (SW) -------------
        # Full-tensor weight loads (768B-contiguous rows; narrower slices DMA
        # ~5x slower), x chunks split 2/2/2 behind wq/wk, RoPE tables after.
        xT_r = xT_d.rearrange("(cc p) t -> p cc t", p=128)
        # gpsimd queue (software DGE, ~3us startup but fastest rate under
        # contention): 3 x chunks + late weights
        nc.gpsimd.dma_start(xt_sb[:, 4, :], xT_r[:, 4, :])
        nc.gpsimd.dma_start(xt_sb[:, 5, :], xT_r[:, 5, :])
        nc.gpsimd.dma_start(xt_sb[:, 2, :], xT_r[:, 2, :])
        nc.gpsimd.dma_start(wv_sb[:], wv_d.rearrange("(cc p) d -> p cc d", p=128))
        nc.gpsimd.dma_start(utri2[:], utri_d)
        nc.gpsimd.dma_start(wo_sb[:], wo_d.rearrange("(hc p) c -> p hc c", p=128))
        # sync queue: rotation matrix (warmup stream), x0 ahead of wq
        # (q0 projects only after k0, so wq can land ~19us), cos table
        nc.sync.dma_start(r2t[:], r2t_d)
        nc.sync.dma_start(xt_sb[:, 0, :], xT_r[:, 0, :])
        nc.sync.dma_start(wq_sb[:], wq_d.rearrange("(cc p) d -> p cc d", p=128))
        nc.sync.dma_start(ccat[:], ccat_d)
        # scalar queue: wk, 2 x chunks, sin table
        nc.scalar.dma_start(wk_sb[:], wk_d.rearrange("(cc p) d -> p cc d", p=128))
        nc.scalar.dma_start(xt_sb[:, 1, :], xT_r[:, 1, :])
        nc.scalar.dma_start(xt_sb[:, 3, :], xT_r[:, 3, :])
        nc.scalar.dma_start(scat[:], scat_d)

        nc.gpsimd.memset(v_aug[:, :, :, D:D + 1], 1.0)

        # x chunk arrival order: x0 (sync, first), x4, x1, then x5, x2, x3
        WAVE1, WAVE2 = (0, 4, 1), (5, 2, 3)

        # ================= phase 1: pair-0 k and q projections =================
        with tc.tile_pool(name="p1ps", bufs=1, space="PSUM") as p1ps, \
             tc.tile_pool(name="p1tmp", bufs=2) as p1tmp:
            # exp table preload + HAM warmup while the first x chunks land
            warm_t = p1ps.tile([128, 512], F32, tag="rot0", name="warm_t")
            warm = warm_t[:, 0:128]
            nc.scalar.activation(exp_warm[:], r2t[0:1, 0:2], EXP)
            for _ in range(60):
                nc.tensor.matmul(warm[:], r2t[:], r2t[:], start=True, stop=True)

            def lead_block2(w_sb, waves, mid=None):
                """Project one [128, T] pair-0 block.  Chunks are consumed in
                (wave, hh) groups so the PSUM->SBUF copy of each half overlaps
                the other half's matmuls; `mid` emits PE work (the previous
                block's rotation) after the second group."""
                ps = [p1ps.tile([128, 1024], F32, tag=f"lead{hh}", name="ps_l")
                      for hh in range(2)]
                qraw = p1tmp.tile([128, T], BF16, tag="qraw")
                seen = {0: 0, 1: 0}
                groups = [(0, waves[0]), (1, waves[0]), (0, waves[1]), (1, waves[1])]
                for gi, (hh, wave) in enumerate(groups):
                    for cc in wave:
                        first = seen[hh] == 0
                        seen[hh] += 1
                        last = seen[hh] == CC
                        for tq in range(2):
                            nc.tensor.matmul(
                                ps[hh][:, tq * 512:(tq + 1) * 512],
                                w_sb[:, cc, 0:128],
                                xt_sb[:, cc,
                                      hh * 1024 + tq * 512:
                                      hh * 1024 + (tq + 1) * 512],
                                start=first, stop=last,
                            )
                    if gi == 1 and mid is not None:
                        mid()
                    if gi >= 2:  # this hh is complete: drain it to SBUF
                        h = hh
                        nc.scalar.copy(qraw[:, h * 1024:(h + 1) * 1024],
                                       ps[h][:, :])
                return qraw

            def rope_finish(dt, dst, qraw):
                """rot matmul + cos/sin combine for a finished raw block.
                The PSUM-reading sin-multiply stays on DVE; the SBUF-only
                cos-multiply and add go to the otherwise idle gpsimd."""
                sin_t = p1tmp.tile([128, T], BF16, tag="sin")
                for hh in range(2):
                    hsl = slice(hh * 1024, (hh + 1) * 1024)
                    ps_r = p1ps.tile([128, 1024], F32, tag=f"rot{hh}", name="ps_r")
                    for tq in range(2):
                        nc.tensor.matmul(
                            ps_r[:, tq * 512:(tq + 1) * 512],
                            r2t[:],
                            qraw[:, hh * 1024 + tq * 512:
                                  hh * 1024 + (tq + 1) * 512],
                            start=True, stop=True,
                        )
                    nc.vector.tensor_mul(sin_t[:, hsl], ps_r[:, :], scat[:, hsl])
                    nc.vector.tensor_mul(dst[:, dt, hsl], qraw[:, hsl],
                                         ccat[:, hsl])
                    nc.vector.tensor_add(dst[:, dt, hsl], dst[:, dt, hsl],
                                         sin_t[:, hsl])

            k0_raw = lead_block2(wk_sb, (WAVE1, WAVE2))
            q0_raw = lead_block2(wq_sb, (WAVE1, WAVE2),
                                 mid=lambda: rope_finish(0, k_all, k0_raw))
            rope_finish(0, q_all, q0_raw)
            # bridge MMs: keep the PE warm while q0's RoPE combine drains on
            # DVE (the first scores depend on it).  Fresh tile in the lead0
            # ring (freed by q0's copy) -- NOT the warm tile, whose slot now
            # belongs to the rope's ps_r ring.
            bridge_t = p1ps.tile([128, 512], F32, tag="lead0", name="bridge_t")
            for _ in range(20):
                nc.tensor.matmul(bridge_t[:, 0:128], r2t[:], r2t[:],
                                 start=True, stop=True)

        # =================== phase 2: fused attention pipeline ===================
        with tc.tile_pool(name="s_ps", bufs=1, space="PSUM") as s_psp, \
             tc.tile_pool(name="pv_ps", bufs=1, space="PSUM") as pv_psp, \
             tc.tile_pool(name="aux_ps", bufs=2, space="PSUM") as aux_psp:

            # ---------- attention building blocks ----------
            def emit_scores_offdiag(p, qc, kcs):
                s_t = [s_psp.tile([128, 1024], F32, tag=f"s{h01}",
                                  name=f"s_t{h01}") for h01 in (0, 1)]
                for j, kc in enumerate(kcs):
                    for h01 in (0, 1):
                        r0, r1 = h01 * 64, h01 * 64 + 64
                        nc.tensor.matmul(
                            s_t[h01][:, j * 512:(j + 1) * 512],
                            k_all[r0:r1, p, kc * 128:(kc + 1) * 128],
                            q_all[r0:r1, p, qc * 512:(qc + 1) * 512],
                            start=True, stop=True,
                        )
                pts = []
                for h01 in (0, 1):
                    pt = p_sbp.tile([128, 1024], BF16, tag=f"pt{h01}")
                    w = len(kcs) * 512
                    nc.scalar.activation(pt[:, 0:w], s_t[h01][:, 0:w], EXP,
                                         scale=0.125)
                    pts.append(pt)
                return pts

            def emit_pv_offdiag(p, qc, kcs, pts, pv):
                for j, kc in enumerate(kcs):
                    for h01 in (0, 1):
                        nc.tensor.matmul(
                            pv[h01][:],
                            v_aug[:, kc, p * 2 + h01, :],
                            pts[h01][:, j * 512:(j + 1) * 512],
                            start=(kc == 0), stop=False,
                        )

            # diagonal tiles: half 0 = j0(512)+j1(384), half 1 = j2(256)+j3(128)
            DIAG_SEGS = (((0, 0, 512), (1, 512, 384)),
                         ((2, 0, 256), (3, 256, 128)))

            def emit_scores_diag(p, qc, segs):
                s_d = [s_psp.tile([128, 1024], F32, tag=f"s{h01}",
                                  name=f"s_d{h01}") for h01 in (0, 1)]
                for j, off, wj in segs:
                    kc = 4 * qc + j
                    for h01 in (0, 1):
                        r0, r1 = h01 * 64, h01 * 64 + 64
                        nc.tensor.matmul(
                            s_d[h01][:, off:off + wj],
                            k_all[r0:r1, p, kc * 128:(kc + 1) * 128],
                            q_all[r0:r1, p, qc * 512 + 128 * j:qc * 512 + 512],
                            start=True, stop=True,
                        )
                pts = []
                for h01 in (0, 1):
                    pt_d = p_sbp.tile([128, 1024], BF16, tag=f"pt{h01}",
                                      name="pt_d")
                    wtot = sum(sg[2] for sg in segs)
                    nc.scalar.activation(pt_d[:, 0:wtot], s_d[h01][:, 0:wtot],
                                         EXP, scale=0.125)
                    # causal keep-mask (utri[k,q] = k<=q) on each seg's leading
                    # 128 cols -- both segs in one strided DVE op (seg pitch
                    # is 512 for half 0, 256 for half 1)
                    pitch = segs[1][1]
                    utri2_v = utri2.rearrange("p (j w) -> p j w", j=2)
                    view = pt_d[:, 0:2 * pitch].rearrange(
                        "p (j w) -> p j w", j=2)[:, :, 0:128]
                    nc.vector.tensor_mul(view, view, utri2_v)
                    pts.append(pt_d)
                return pts

            def emit_pv_diag(p, qc, segs, pts, pv, last):
                for j, off, wj in segs:
                    kc = 4 * qc + j
                    for h01 in (0, 1):
                        nc.tensor.matmul(
                            pv[h01][:, 128 * j:512],
                            v_aug[:, kc, p * 2 + h01, :],
                            pts[h01][:, off:off + wj],
                            start=(kc == 0), stop=(last and j == 3),
                        )

            def norm_copy(p, qc, pv, tail=False):
                """Drain pv into SBUF fast (frees the PSUM slots) and kick off
                the gpsimd broadcast; the reciprocal+multiply finish is
                deferred into the next stretch."""
                ovs, rbcs = [], []
                for h01 in (0, 1):
                    lrow = l_sbp.tile([1, 512], F32, tag=f"l{h01}")
                    ov = l_sbp.tile([64, 512], BF16, tag=f"ov{h01}", name="ov")
                    if tail:
                        nc.scalar.copy(lrow[0:1, :], pv[h01][64:65, :])
                        nc.scalar.copy(ov[:], pv[h01][0:64, :])
                    else:
                        nc.vector.tensor_copy(lrow[0:1, :], pv[h01][64:65, :])
                        nc.vector.tensor_copy(ov[:], pv[h01][0:64, :])
                    rbc = r_sbp.tile([64, 512], F32, tag=f"r{h01}")
                    nc.gpsimd.partition_broadcast(rbc[:], lrow[0:1, :],
                                                  channels=64)
                    ovs.append(ov)
                    rbcs.append(rbc)

                def finish():
                    for h01 in (0, 1):
                        nc.vector.reciprocal_approx_fast(rbcs[h01][:],
                                                         rbcs[h01][:])
                        nc.vector.tensor_mul(
                            out_norm[h01 * 64:h01 * 64 + 64, p,
                                     qc * 512:(qc + 1) * 512],
                            ovs[h01][:],
                            rbcs[h01][:],
                        )

                return finish

            def attn_units(p, qc):
                units = []
                for g0 in range(0, 4 * qc, 2):
                    kcs = list(range(g0, min(g0 + 2, 4 * qc)))
                    units.append((
                        (lambda kk: lambda: emit_scores_offdiag(p, qc, kk))(kcs),
                        (lambda kk: lambda pts, pv: emit_pv_offdiag(
                            p, qc, kk, pts, pv))(kcs),
                    ))
                for half, segs in enumerate(DIAG_SEGS):
                    units.append((
                        (lambda ss: lambda: emit_scores_diag(p, qc, ss))(segs),
                        (lambda ss, la: lambda pts, pv: emit_pv_diag(
                            p, qc, ss, pts, pv, last=la))(segs, half == 1),
                    ))
                return units

            # ---------- PE filler work ----------
            def emit_vproj(tt):
                ps_v = aux_psp.tile([128, HPC * D], F32, tag="aux", name="ps_v")
                for cc in range(CC):
                    nc.tensor.matmul(
                        ps_v[:, 0:HPC * D],
                        xt_sb[:, cc, tt * 128:(tt + 1) * 128],
                        wv_sb[:, cc, :],
                        start=(cc == 0), stop=(cc == CC - 1),
                    )
                nc.vector.tensor_copy(
                    v_aug[:, tt, :, 0:D],
                    ps_v[:, 0:HPC * D].rearrange("p (h d) -> p h d", d=D),
                )

            def proj_quarter_fns(w_sb, p, dst):
                """Project+RoPE one later (pair p) block as 8 interleaved
                fillers: 4 projection quarters and 4 rotation quarters, each
                cycling through the 1-bank aux PSUM ring.  Rotations trail
                their projection by one filler slot so the PSUM->SBUF copy
                latency hides under attention matmuls."""
                state = {}

                def proj_q(qq):
                    sl = slice(qq * 512, (qq + 1) * 512)
                    qraw = qr_sbp.tile([128, 512], BF16, tag="qr", name="qraw_f")
                    ps_q = aux_psp.tile([128, 512], F32, tag="aux", name="ps_q")
                    for cc in range(CC):
                        nc.tensor.matmul(
                            ps_q[:],
                            w_sb[:, cc, p * 128:(p + 1) * 128],
                            xt_sb[:, cc, sl],
                            start=(cc == 0), stop=(cc == CC - 1),
                        )
                    nc.vector.tensor_copy(qraw[:], ps_q[:])
                    state[qq] = qraw

                def rot_q(qq):
                    sl = slice(qq * 512, (qq + 1) * 512)
                    qraw = state[qq]
                    sin_q = qr_sbp.tile([128, 512], BF16, tag="sin", name="sin_f")
                    ps_r = aux_psp.tile([128, 512], F32, tag="aux", name="ps_rf")
                    nc.tensor.matmul(ps_r[:], r2t[:], qraw[:],
                                     start=True, stop=True)
                    nc.vector.tensor_mul(sin_q[:], ps_r[:], scat[:, sl])
                    nc.vector.tensor_mul(dst[:, p, sl], qraw[:], ccat[:, sl])
                    nc.vector.tensor_add(dst[:, p, sl], dst[:, p, sl], sin_q[:])

                fns = []
                for qq in range(4):
                    fns.append((lambda q: lambda: proj_q(q))(qq))
                    if qq >= 1:
                        fns.append((lambda q: lambda: rot_q(q))(qq - 1))
                fns.append(lambda: rot_q(3))
                return fns

            def emit_outproj_tile(tt, tail=False):
                y_a = aux_psp.tile([128, 384], F32, tag="aux", name="y_a")
                y_b = aux_psp.tile([128, 384], F32, tag="aux", name="y_b")
                for hc in range(NP):
                    lhsT = out_norm[:, hc, tt * 128:(tt + 1) * 128]
                    nc.tensor.matmul(y_a[:, 0:384], lhsT,
                                     wo_sb[:, hc, 0:384],
                                     start=(hc == 0), stop=(hc == NP - 1))
                    nc.tensor.matmul(y_b[:, 0:384], lhsT,
                                     wo_sb[:, hc, 384:768],
                                     start=(hc == 0), stop=(hc == NP - 1))
                yt = y_sbp.tile([128, C], BF16, tag="yt")
                if tail:
                    nc.scalar.copy(yt[:, 0:384], y_a[:, 0:384])
                    nc.scalar.copy(yt[:, 384:768], y_b[:, 0:384])
                else:
                    nc.vector.tensor_copy(yt[:, 0:384], y_a[:, 0:384])
                    nc.vector.tensor_copy(yt[:, 384:768], y_b[:, 0:384])
                if tail:
                    eng = (nc.scalar, nc.gpsimd, nc.sync, nc.scalar)[tt % 4]
                else:
                    eng = (nc.sync, nc.gpsimd)[tt % 2]
                eng.dma_start(y_d[tt * 128:(tt + 1) * 128, :], yt[:])

            def outproj_fns(qc):
                return [(lambda t: lambda: emit_outproj_tile(t))(tt)
                        for tt in range(4 * qc, 4 * qc + 4)]

            pending = []

            def emit_attn(p, qc, pv, fillers=None, fill_from=0):
                """Ping-pong with 1-group score lookahead; filler closures are
                paced evenly across the stretch's units (starting at unit
                fill_from); the previous stretch's deferred norm-finish is
                flushed after unit 1."""
                fillers = list(fillers or [])
                fi = 0
                units = attn_units(p, qc)
                nu = len(units)
                queue = []
                for si, (sc_fn, pv_fn) in enumerate(units):
                    queue.append((pv_fn, sc_fn()))
                    if len(queue) > 1:
                        fn, pts = queue.pop(0)
                        fn(pts, pv)
                    if si == 1:
                        while pending:
                            pending.pop(0)()
                    denom = max(1, nu - fill_from)
                    prog = si + 1 - fill_from
                    target = (0 if prog <= 0
                              else (len(fillers) * prog + denom - 1) // denom)
                    while fi < len(fillers) and fi < target:
                        fillers[fi]()
                        fi += 1
                while fi < len(fillers):
                    fillers[fi]()
                    fi += 1
                for fn, pts in queue:
                    fn(pts, pv)
                pending.append(norm_copy(p, qc, pv, tail=(qc == 0)))

            def new_pv():
                return [pv_psp.tile([65, 512], F32, tag=f"pv{h01}",
                                    name=f"pv{h01}") for h01 in (0, 1)]

            # ---------------- the stretch schedule ----------------
            vproj_fillers = [(lambda t: lambda: emit_vproj(t))(tt)
                             for tt in range(KC)]
            emit_attn(0, 3, new_pv(), fillers=vproj_fillers)
            emit_attn(0, 2, new_pv(), fillers=proj_quarter_fns(wq_sb, 1, q_all))
            emit_attn(0, 1, new_pv(), fillers=proj_quarter_fns(wk_sb, 1, k_all))
            emit_attn(1, 3, new_pv(), fillers=proj_quarter_fns(wq_sb, 2, q_all))
            emit_attn(1, 2, new_pv(), fillers=proj_quarter_fns(wk_sb, 2, k_all))
            emit_attn(2, 3, new_pv())
            emit_attn(2, 2, new_pv(), fillers=outproj_fns(3), fill_from=3)
            emit_attn(1, 1, new_pv(), fillers=outproj_fns(2), fill_from=3)
            emit_attn(2, 1, new_pv())

            # final two qc0 stretches: scores+exps hoisted ahead of the
            # (0,0) stretch, PVs+norms breadth-first after it
            helds = [[(pv_fn, sc_fn()) for sc_fn, pv_fn in attn_units(p, 0)]
                     for p in (1, 2)]
            emit_attn(0, 0, new_pv(), fillers=outproj_fns(1), fill_from=1)
            while pending:
                pending.pop(0)()          # nf(0,0) as early as possible
            for pi, p in enumerate((1, 2)):
                pv = new_pv()
                for pv_fn, pts in helds[pi]:
                    pv_fn(pts, pv)
                norm_copy(p, 0, pv, tail=True)()  # finish immediately
            # outproj(0): the score-tag PSUM slots are dead after the last
            # exp, so borrow them ([128,1024] = 2 banks; halves at the
            # bank-aligned cols 0 and 512) to keep two y tiles in flight,
            # with per-hc matmuls pipelining against the qc0 norm chains
            for t0 in (0, 2):
                ys = []
                for tt in (t0, t0 + 1):
                    yp = s_psp.tile([128, 1024], F32, tag=f"s{tt - t0}",
                                    name="y_ps")
                    ys.append((tt, yp))
                for hc in range(NP):
                    for tt, yp in ys:
                        lhsT = out_norm[:, hc, tt * 128:(tt + 1) * 128]
                        nc.tensor.matmul(yp[:, 0:384], lhsT,
                                         wo_sb[:, hc, 0:384],
                                         start=(hc == 0), stop=(hc == NP - 1))
                        nc.tensor.matmul(yp[:, 512:896], lhsT,
                                         wo_sb[:, hc, 384:768],
                                         start=(hc == 0), stop=(hc == NP - 1))
                for tt, yp in ys:
                    yt = y_sbp.tile([128, C], BF16, tag="yt")
                    nc.scalar.copy(yt[:, 0:384], yp[:, 0:384])
                    nc.scalar.copy(yt[:, 384:768], yp[:, 512:896])
                    eng = (nc.scalar, nc.gpsimd, nc.sync, nc.scalar)[tt % 4]
                    eng.dma_start(y_d[tt * 128:(tt + 1) * 128, :], yt[:])

    nc.compile()
    return nc


# make mybir importable inside _build_program's nested scopes
from concourse import mybir  # noqa: E402


def _get_compiled():
    global _COMPILED
    if _COMPILED is None:
        _COMPILED = _build_program()
    return _COMPILED


def _make_in_maps(inputs):
    import ml_dtypes

    BF = ml_dtypes.bfloat16
    x = np.asarray(inputs["x"], dtype=np.float32)
    wq = np.asarray(inputs["wq"], dtype=np.float32).astype(BF)
    wk = np.asarray(inputs["wk"], dtype=np.float32).astype(BF)
    wv = np.asarray(inputs["wv"], dtype=np.float32).astype(BF)
    wo = np.asarray(inputs["wo"], dtype=np.float32).astype(BF)

    ccat, scat = _rope_tables()
    r2t = _rot_matrix()
    m = np.arange(128)
    utri1 = (m[:, None] <= m[None, :]).astype(BF)
    utri = np.ascontiguousarray(np.concatenate([utri1, utri1], axis=1))

    xTs = [np.ascontiguousarray(x[b].T.astype(BF)) for b in range(B)]
    in_maps = []
    for c in range(8):
        b, g = c // 2, c % 2
        sl = slice(g * HPC * D, (g + 1) * HPC * D)
        in_maps.append(dict(
            xT=xTs[b],
            wq=np.ascontiguousarray(wq[:, sl]),
            wk=np.ascontiguousarray(wk[:, sl]),
            wv=np.ascontiguousarray(wv[:, sl]),
            wo=np.ascontiguousarray(wo[sl, :]),
            ccat=ccat, scat=scat, r2t=r2t, utri=utri,
        ))
    return in_maps


def kernel(x, wq, wk, wv, wo, mask):
    """Full inputs in, full output out. Shards across 8 NeuronCores internally.

    The mask input is the standard causal mask produced by setup_inputs();
    causality is implemented directly on-device.
    """
    from concourse.bass_utils import run_bass_kernel_spmd

    in_maps = _make_in_maps(dict(x=x, wq=wq, wk=wk, wv=wv, wo=wo))

    nc = _get_compiled()
    res = run_bass_kernel_spmd(nc, in_maps, list(range(8)))
    out = np.empty((B, T, C), dtype=np.float32)
    for b in range(B):
        out[b] = (res.results[2 * b]["y"].astype(np.float32)
                  + res.results[2 * b + 1]["y"].astype(np.float32))
    return out
